# revision 14
# baseline (speedup 1.0000x reference)
"""Trainium2 Bass kernel for gather + segment-sum message passing.

out = segment_sum(x[index_j], index_i, num_segments=N)

Strategy (no collectives): shard the OUTPUT (node) dimension across the
8 cores (12500 nodes each); x replicated in DRAM.  Host-side prep per
core:
  - triplets bucketed by (src chunk c of 25000 rows, dest superblock sb
    of 256 nodes), each (c,sb) cell padded to a multiple of 128 slots
    (uniform across cores so the SPMD program is shape-identical),
  - gather indices are chunk-local (fit int16 for the dma_gather
    custom op; pad slots point at row 0 of the chunk),
  - r[slot] = in-superblock destination offset (0..255) as bf16, -1
    for pads.
Slot layout is group-major -> chunk -> superblock, so each (group,
chunk) is one contiguous slot range = ONE dma_gather call.  The Q7
SWDGE descriptor-generation (~8.4 ns/index) is the main cost; the 4
SWDGE queues map to disjoint Q7 cpu pairs, so per-chunk gathers go on
queue c and desc-gen runs 4-way parallel.
On device, per superblock: a batched bf16 is_equal against an iota row
builds one-hot selection matrices S[msg, 256 node slots] (pad slots
match nothing); TensorE scatter-adds psum_q[node, f] += S_q.T @ msg
for the 2 psum halves, accumulated over the superblock's tiles;
ScalarE copies PSUM to an SBUF staging buffer; one contiguous DRAM
store at the end (host undoes the layout permutation).
Messages are gathered from a hi/lo bf16-split copy of x (one 256-byte
row per index); only the hi half feeds the matmul (~4e-3 relative
accuracy, well within the 2e-2 gate); PSUM accumulates in fp32.
"""

import numpy as np

N_NODES = 100000
N_TRIPLETS = 1250000
F = 64
N_CORES = 8
N_LOC = N_NODES // N_CORES  # 12500 nodes per core
P = 128
SB = 256                    # dest superblock (nodes)
N_SB = (N_LOC + SB - 1) // SB   # 49 superblocks per core
N_BLK = N_SB * (SB // P)        # 98 psum blocks of 128
N_CHUNK = 4
CHUNK_ROWS = N_NODES // N_CHUNK  # 25000 src rows per chunk
G_SB = 4                    # superblocks per gather-call group

_PROG_CACHE = {}

GROUPS = [list(range(g0, min(g0 + G_SB, N_SB)))
          for g0 in range(0, N_SB, G_SB)]


def _cell_order():
    """Canonical cell enumeration shared by host prep and builder:
    group-major -> chunk -> superblock."""
    cells = []
    for sbs in GROUPS:
        for c in range(N_CHUNK):
            for s in sbs:
                cells.append((c, s))
    return cells


def _build_program(t_csb):
    """t_csb: [N_CHUNK][N_SB] message-tile counts per cell (uniform
    across cores by construction)."""
    import concourse.tile as tile
    from concourse import bacc, mybir

    fdt = mybir.dt.float32
    bdt = mybir.dt.bfloat16
    idt16 = mybir.dt.int16
    mcols = 2 * F  # hi|lo bf16 row = 256 bytes

    cells = _cell_order()
    tile_off = {}
    run = 0
    for (c, s) in cells:
        tile_off[(c, s)] = run
        run += t_csb[c][s]
    ktot = run
    n_slots = ktot * P
    ntmax = max(max(row) for row in t_csb)

    nc = bacc.Bacc("TRN2", target_bir_lowering=False, debug=False,
                   num_devices=1, num_swdge_queues=4)

    x_ap = nc.dram_tensor("x", [N_NODES, mcols], bdt,
                          kind="ExternalInput").ap()
    idx_ap = nc.dram_tensor("idx_w", [P, n_slots // 16], idt16,
                            kind="ExternalInput").ap()
    r_ap = nc.dram_tensor("r_t", [P, ktot], bdt, kind="ExternalInput").ap()
    # iota_rep[p, n*ntmax + t] = n : per-tile-broadcast iota so the
    # is_equal runs with packed innermost APs (DVE 2x_1p mode)
    iota_ap = nc.dram_tensor("iota", [P, SB * ntmax], bdt,
                             kind="ExternalInput").ap()
    out_ap = nc.dram_tensor("out_t", [P, N_BLK * F], fdt,
                            kind="ExternalOutput").ap()

    with tile.TileContext(nc) as tc:
        with (
            tc.tile_pool(name="res", bufs=1) as res_pool,
            tc.tile_pool(name="stage", bufs=2) as stage_pool,
            tc.tile_pool(name="msg", bufs=4) as msg_pool,
            tc.tile_pool(name="sel", bufs=12) as sel_pool,
            tc.tile_pool(name="psum", bufs=3, space="PSUM") as psum_pool,
        ):
            r_sb = res_pool.tile([P, ktot], bdt)
            nc.sync.dma_start(r_sb[:], r_ap[:])
            iota_sb = res_pool.tile([P, SB * ntmax], bdt)
            nc.sync.dma_start(iota_sb[:], iota_ap[:])
            # per-group idx loads so the first gather starts early
            idx_g = []
            for gi, sbs in enumerate(GROUPS):
                k0 = tile_off[(0, sbs[0])]
                k1 = (tile_off[(3, sbs[-1])] + t_csb[3][sbs[-1]])
                t_ig = res_pool.tile([P, (k1 - k0) * 8], idt16,
                                     name=f"idx_{gi}")
                nc.sync.dma_start(t_ig[:], idx_ap[:, k0 * 8:k1 * 8])
                idx_g.append((t_ig, k0))

            for gi, sbs in enumerate(GROUPS):
                # one gather call per (group, chunk): the group's cells
                # for that chunk are contiguous in slot space
                idx_sb, gk0 = idx_g[gi]
                gmsg = {}
                for c in range(N_CHUNK):
                    ct = sum(t_csb[c][s] for s in sbs)
                    if ct == 0:
                        continue
                    g0 = tile_off[(c, sbs[0])]
                    msg = msg_pool.tile([P, ct * mcols], bdt, tag=f"msg{c}",
                                        name=f"msg_{gi}_{c}")
                    nc.gpsimd.dma_gather(
                        msg[:].rearrange("p (t e) -> p t e", e=mcols),
                        x_ap[c * CHUNK_ROWS:(c + 1) * CHUNK_ROWS, :],
                        idx_sb[:, (g0 - gk0) * 8:(g0 - gk0 + ct) * 8],
                        ct * P,
                        ct * P,
                        mcols,
                        single_packet=False,
                        queue_num=c,
                    )
                    gmsg[c] = (msg, g0)

                blk0 = sbs[0] * (SB // P)
                nblk_g = len(sbs) * (SB // P)
                out_g = stage_pool.tile([P, nblk_g * F], fdt, tag="out",
                                        name=f"out_{gi}")
                for s in sbs:
                    n_tiles_sb = sum(t_csb[c][s] for c in range(N_CHUNK))
                    if n_tiles_sb == 0:
                        continue
                    sels = {}
                    for c in range(N_CHUNK):
                        nt = t_csb[c][s]
                        if nt == 0:
                            continue
                        g0 = tile_off[(c, s)]
                        sel = sel_pool.tile([P, SB * nt], bdt,
                                            tag="sel", name=f"sel_{s}_{c}")
                        # S[q, n, t] = (r[q, t] == n), n in [0, 256);
                        # layout [P, SB, nt]: innermost t is packed on all
                        # operands -> DVE 2x_1p fast mode
                        nc.vector.tensor_tensor(
                            out=sel[:].rearrange("p (n t) -> p n t", t=nt),
                            in0=r_sb[:, None, g0:g0 + nt].to_broadcast(
                                [P, SB, nt]),
                            in1=iota_sb[:].rearrange(
                                "p (n t) -> p n t", t=ntmax)[:, :, :nt],
                            op=mybir.AluOpType.is_equal,
                        )
                        sels[c] = sel

                    psums = [psum_pool.tile([P, F], fdt, tag=f"acc{q}",
                                            name=f"ps_{s}_{q}")
                             for q in range(SB // P)]
                    mm_i = 0
                    n_mm = n_tiles_sb
                    for c in range(N_CHUNK):
                        nt = t_csb[c][s]
                        if nt == 0:
                            continue
                        msg, mg0 = gmsg[c]
                        lt0 = tile_off[(c, s)] - mg0
                        sel = sels[c]
                        sel_r = sel[:].rearrange("p (n t) -> p n t", t=nt)
                        for t in range(nt):
                            mt = lt0 + t
                            for q in range(SB // P):
                                lhs = sel_r[:, q * P:(q + 1) * P, t]
                                nc.tensor.matmul(
                                    out=psums[q][:],
                                    lhsT=lhs,
                                    rhs=msg[:, mt * mcols:mt * mcols + F],
                                    start=(mm_i == 0),
                                    stop=(mm_i == n_mm - 1),
                                )
                            mm_i += 1
                    for q in range(SB // P):
                        blk = s * (SB // P) + q - blk0
                        nc.scalar.copy(out_g[:, blk * F:(blk + 1) * F],
                                       psums[q][:])
                nc.sync.dma_start(
                    out_ap[:, blk0 * F:(blk0 + nblk_g) * F], out_g[:])

    nc.compile()
    return nc, ktot, n_slots


def _host_prep(x, triplet_indices):
    j = np.ascontiguousarray(triplet_indices[:, 1]).astype(np.int64)
    i = np.ascontiguousarray(triplet_indices[:, 2]).astype(np.int64)

    core = i // N_LOC
    d = i - core * N_LOC            # local dest node, 0..12499
    sb = d >> 8                     # dest superblock, 0..48
    c = j // CHUNK_ROWS             # src chunk, 0..3
    jl = (j - c * CHUNK_ROWS).astype(np.int16)

    cells = _cell_order()
    n_cell = len(cells)
    cellrank = np.empty((N_CHUNK, N_SB), dtype=np.int64)
    for rank, (cc, ss) in enumerate(cells):
        cellrank[cc, ss] = rank

    key = core * n_cell + cellrank[c, sb]
    order = np.argsort(key, kind="stable")
    key_s = key[order]
    jl_s = jl[order]
    r_s = (d[order] & (SB - 1)).astype(np.int16)

    n_groups = N_CORES * n_cell
    counts = np.bincount(key_s, minlength=n_groups).reshape(
        N_CORES, n_cell)
    cell_max = counts.max(axis=0)             # per cell rank
    t_cell = -(-cell_max // P)                # tiles per cell
    slots_cell = t_cell * P

    off_cell = np.zeros(n_cell + 1, dtype=np.int64)
    np.cumsum(slots_cell, out=off_cell[1:])
    n_slots = int(off_cell[-1])
    ktot = n_slots // P

    starts = np.zeros(n_groups, dtype=np.int64)
    np.cumsum(counts.ravel()[:-1], out=starts[1:])
    within = np.arange(len(key_s), dtype=np.int64) - starts[key_s]
    cell_of = key_s % n_cell
    slot = off_cell[cell_of] + within
    core_s = key_s // n_cell

    src_pad = np.zeros((N_CORES, n_slots), dtype=np.int16)  # pad -> row 0
    r_pad = np.full((N_CORES, n_slots), -1, dtype=np.int16)
    src_pad[core_s, slot] = jl_s
    r_pad[core_s, slot] = r_s

    idx_w = src_pad.reshape(N_CORES, n_slots // 16, 16).transpose(0, 2, 1)
    idx_w = np.ascontiguousarray(np.tile(idx_w, (1, 8, 1)))

    import ml_dtypes
    r_t = np.ascontiguousarray(
        r_pad.reshape(N_CORES, ktot, P).transpose(0, 2, 1)).astype(
            ml_dtypes.bfloat16)

    ntmax = int(t_cell.max())
    iota = np.broadcast_to(
        np.repeat(np.arange(SB), ntmax).astype(ml_dtypes.bfloat16),
        (P, SB * ntmax)).copy()

    hi = x.astype(ml_dtypes.bfloat16)
    lo = (x.astype(np.float32) - hi.astype(np.float32)).astype(
        ml_dtypes.bfloat16)
    x_dev = np.ascontiguousarray(np.concatenate([hi, lo], axis=1))

    t_csb = np.zeros((N_CHUNK, N_SB), dtype=np.int64)
    for rank, (cc, ss) in enumerate(cells):
        t_csb[cc, ss] = t_cell[rank]

    in_maps = [
        {"x": x_dev, "idx_w": idx_w[c2], "r_t": r_t[c2], "iota": iota}
        for c2 in range(N_CORES)
    ]
    return in_maps, t_csb, n_slots


def kernel(x, triplet_indices, _msg_dt="bf16hl", _return_nc=False):
    x = np.asarray(x)
    triplet_indices = np.asarray(triplet_indices)

    in_maps, t_csb, n_slots = _host_prep(x, triplet_indices)

    cache_key = t_csb.tobytes()
    if cache_key not in _PROG_CACHE:
        _PROG_CACHE[cache_key] = _build_program(t_csb.tolist())
    nc, ktot, n_slots2 = _PROG_CACHE[cache_key]
    assert n_slots2 == n_slots

    from concourse.bass_utils import run_bass_kernel_spmd

    res = run_bass_kernel_spmd(nc, in_maps, core_ids=list(range(N_CORES)))

    out = np.empty((N_NODES, F), dtype=np.float32)
    for c2 in range(N_CORES):
        o = res.results[c2]["out_t"]  # [128, N_BLK*F]
        o = o.reshape(P, N_BLK, F).transpose(1, 0, 2).reshape(N_BLK * P, F)
        out[c2 * N_LOC:(c2 + 1) * N_LOC] = o[:N_LOC]
    if _return_nc:
        return out, nc, in_maps
    return out


# revision 18
# speedup vs baseline: 1.1460x; 1.1460x over previous
"""Trainium2 Bass kernel for gather + segment-sum message passing.

out = segment_sum(x[index_j], index_i, num_segments=N)

Strategy (no collectives): shard the OUTPUT (node) dimension across the
8 cores (12500 nodes each); x replicated in DRAM.  Host-side prep per
core:
  - triplets bucketed by (src chunk c of 25000 rows, dest superblock sb
    of 256 nodes), each (c,sb) cell padded to a multiple of 128 slots
    (uniform across cores so the SPMD program is shape-identical),
  - gather indices are chunk-local (fit int16 for the dma_gather
    custom op; pad slots point at row 0 of the chunk),
  - r[slot] = in-superblock destination offset (0..255) as bf16, -1
    for pads.
Slot layout is group-major -> chunk -> superblock, so each (group,
chunk) is one contiguous slot range = ONE dma_gather call.  The Q7
SWDGE descriptor-generation (~8.4 ns/index) is the main cost; the 4
SWDGE queues map to disjoint Q7 cpu pairs, so per-chunk gathers go on
queue c and desc-gen runs 4-way parallel.
On device, per superblock: a batched bf16 is_equal against an iota row
builds one-hot selection matrices S[msg, 256 node slots] (pad slots
match nothing); TensorE scatter-adds psum_q[node, f] += S_q.T @ msg
for the 2 psum halves, accumulated over the superblock's tiles;
ScalarE copies PSUM to an SBUF staging buffer; one contiguous DRAM
store at the end (host undoes the layout permutation).
Messages are gathered from a hi/lo bf16-split copy of x (one 256-byte
row per index); only the hi half feeds the matmul (~4e-3 relative
accuracy, well within the 2e-2 gate); PSUM accumulates in fp32.
"""

import numpy as np

N_NODES = 100000
N_TRIPLETS = 1250000
F = 64
N_CORES = 8
N_LOC = N_NODES // N_CORES  # 12500 nodes per core
P = 128
SB = 256                    # dest superblock (nodes)
N_SB = 52                   # superblocks per core (load-balanced bins)
N_BLK = N_SB * (SB // P)        # 104 psum blocks of 128
N_CHUNK = 4
CHUNK_ROWS = N_NODES // N_CHUNK  # 25000 src rows per chunk
G_SB = 4                    # superblocks per gather-call group

_PROG_CACHE = {}

GROUPS = [list(range(g0, min(g0 + G_SB, N_SB)))
          for g0 in range(0, N_SB, G_SB)]


def _cell_order():
    """Canonical cell enumeration shared by host prep and builder:
    group-major -> chunk -> superblock."""
    cells = []
    for sbs in GROUPS:
        for c in range(N_CHUNK):
            for s in sbs:
                cells.append((c, s))
    return cells


def _build_program(t_csb):
    """t_csb: [N_CHUNK][N_SB] message-tile counts per cell (uniform
    across cores by construction)."""
    import concourse.tile as tile
    from concourse import bacc, mybir

    fdt = mybir.dt.float32
    bdt = mybir.dt.bfloat16
    idt16 = mybir.dt.int16
    mcols = 2 * F  # hi|lo bf16 row = 256 bytes

    cells = _cell_order()
    tile_off = {}
    run = 0
    for (c, s) in cells:
        tile_off[(c, s)] = run
        run += t_csb[c][s]
    ktot = run
    n_slots = ktot * P
    ntmax = max(max(row) for row in t_csb)

    nc = bacc.Bacc("TRN2", target_bir_lowering=False, debug=False,
                   num_devices=1, num_swdge_queues=4)

    x_ap = nc.dram_tensor("x", [N_NODES, mcols], bdt,
                          kind="ExternalInput").ap()
    idx_ap = nc.dram_tensor("idx_w", [P, n_slots // 16], idt16,
                            kind="ExternalInput").ap()
    r_ap = nc.dram_tensor("r_t", [P, ktot], bdt, kind="ExternalInput").ap()
    # iota_rep[p, n*ntmax + t] = n : per-tile-broadcast iota so the
    # is_equal runs with packed innermost APs (DVE 2x_1p mode)
    iota_ap = nc.dram_tensor("iota", [P, SB * ntmax], bdt,
                             kind="ExternalInput").ap()
    out_ap = nc.dram_tensor("out_t", [P, N_BLK * F], fdt,
                            kind="ExternalOutput").ap()

    with tile.TileContext(nc) as tc:
        with (
            tc.tile_pool(name="res", bufs=1) as res_pool,
            tc.tile_pool(name="stage", bufs=2) as stage_pool,
            tc.tile_pool(name="msg", bufs=4) as msg_pool,
            tc.tile_pool(name="sel", bufs=12) as sel_pool,
            tc.tile_pool(name="psum", bufs=3, space="PSUM") as psum_pool,
        ):
            r_sb = res_pool.tile([P, ktot], bdt)
            nc.sync.dma_start(r_sb[:], r_ap[:])
            iota_sb = res_pool.tile([P, SB * ntmax], bdt)
            nc.sync.dma_start(iota_sb[:], iota_ap[:])
            # per-group idx loads so the first gather starts early
            idx_g = []
            for gi, sbs in enumerate(GROUPS):
                k0 = tile_off[(0, sbs[0])]
                k1 = (tile_off[(3, sbs[-1])] + t_csb[3][sbs[-1]])
                t_ig = res_pool.tile([P, (k1 - k0) * 8], idt16,
                                     name=f"idx_{gi}")
                nc.sync.dma_start(t_ig[:], idx_ap[:, k0 * 8:k1 * 8])
                idx_g.append((t_ig, k0))

            for gi, sbs in enumerate(GROUPS):
                # one gather call per (group, chunk): the group's cells
                # for that chunk are contiguous in slot space
                idx_sb, gk0 = idx_g[gi]
                gmsg = {}
                for c in range(N_CHUNK):
                    ct = sum(t_csb[c][s] for s in sbs)
                    if ct == 0:
                        continue
                    g0 = tile_off[(c, sbs[0])]
                    msg = msg_pool.tile([P, ct * mcols], bdt, tag=f"msg{c}",
                                        name=f"msg_{gi}_{c}")
                    nc.gpsimd.dma_gather(
                        msg[:].rearrange("p (t e) -> p t e", e=mcols),
                        x_ap[c * CHUNK_ROWS:(c + 1) * CHUNK_ROWS, :],
                        idx_sb[:, (g0 - gk0) * 8:(g0 - gk0 + ct) * 8],
                        ct * P,
                        ct * P,
                        mcols,
                        single_packet=False,
                        queue_num=c,
                    )
                    gmsg[c] = (msg, g0)

                blk0 = sbs[0] * (SB // P)
                nblk_g = len(sbs) * (SB // P)
                out_g = stage_pool.tile([P, nblk_g * F], fdt, tag="out",
                                        name=f"out_{gi}")
                for s in sbs:
                    n_tiles_sb = sum(t_csb[c][s] for c in range(N_CHUNK))
                    if n_tiles_sb == 0:
                        continue
                    sels = {}
                    for c in range(N_CHUNK):
                        nt = t_csb[c][s]
                        if nt == 0:
                            continue
                        g0 = tile_off[(c, s)]
                        sel = sel_pool.tile([P, SB * nt], bdt,
                                            tag="sel", name=f"sel_{s}_{c}")
                        # S[q, n, t] = (r[q, t] == n), n in [0, 256);
                        # layout [P, SB, nt]: innermost t is packed on all
                        # operands -> DVE 2x_1p fast mode
                        nc.vector.tensor_tensor(
                            out=sel[:].rearrange("p (n t) -> p n t", t=nt),
                            in0=r_sb[:, None, g0:g0 + nt].to_broadcast(
                                [P, SB, nt]),
                            in1=iota_sb[:].rearrange(
                                "p (n t) -> p n t", t=ntmax)[:, :, :nt],
                            op=mybir.AluOpType.is_equal,
                        )
                        sels[c] = sel

                    psums = [psum_pool.tile([P, F], fdt, tag=f"acc{q}",
                                            name=f"ps_{s}_{q}")
                             for q in range(SB // P)]
                    mm_i = 0
                    n_mm = n_tiles_sb
                    for c in range(N_CHUNK):
                        nt = t_csb[c][s]
                        if nt == 0:
                            continue
                        msg, mg0 = gmsg[c]
                        lt0 = tile_off[(c, s)] - mg0
                        sel = sels[c]
                        sel_r = sel[:].rearrange("p (n t) -> p n t", t=nt)
                        for t in range(nt):
                            mt = lt0 + t
                            for q in range(SB // P):
                                lhs = sel_r[:, q * P:(q + 1) * P, t]
                                nc.tensor.matmul(
                                    out=psums[q][:],
                                    lhsT=lhs,
                                    rhs=msg[:, mt * mcols:mt * mcols + F],
                                    start=(mm_i == 0),
                                    stop=(mm_i == n_mm - 1),
                                )
                            mm_i += 1
                    for q in range(SB // P):
                        blk = s * (SB // P) + q - blk0
                        nc.scalar.copy(out_g[:, blk * F:(blk + 1) * F],
                                       psums[q][:])
                nc.sync.dma_start(
                    out_ap[:, blk0 * F:(blk0 + nblk_g) * F], out_g[:])

    nc.compile()
    return nc, ktot, n_slots


def _assign_bins(i, c):
    """Greedy chunk-aware balancing of dest nodes into the
    N_CORES*N_SB (core, superblock) bins of <=SB dests each, so every
    (bin, chunk) cell count stays ~T/(bins*chunks) and the uniform
    per-cell tile count is minimal.  Returns (bin_of, r_of): for each
    dest node its bin id and its position within the bin."""
    nbins = N_CORES * N_SB
    cnt = np.bincount(i * N_CHUNK + c,
                      minlength=N_NODES * N_CHUNK).reshape(
        N_NODES, N_CHUNK).astype(np.int64)
    order = np.argsort(-cnt.sum(1), kind="stable")
    loads = np.zeros((nbins, N_CHUNK), dtype=np.int64)
    fill = np.zeros(nbins, dtype=np.int64)
    bin_of = np.empty(N_NODES, dtype=np.int64)
    r_of = np.empty(N_NODES, dtype=np.int64)
    for dest in order:
        v = cnt[dest]
        score = (loads + v).max(1) + (fill == SB) * (1 << 30)
        b = int(score.argmin())
        bin_of[dest] = b
        r_of[dest] = fill[b]
        fill[b] += 1
        loads[b] += v
    return bin_of, r_of


def _host_prep(x, triplet_indices):
    j = np.ascontiguousarray(triplet_indices[:, 1]).astype(np.int64)
    i = np.ascontiguousarray(triplet_indices[:, 2]).astype(np.int64)

    c = j // CHUNK_ROWS             # src chunk, 0..3
    jl = (j - c * CHUNK_ROWS).astype(np.int16)

    bin_of, r_of = _assign_bins(i, c)
    core = bin_of[i] // N_SB
    sb = bin_of[i] - core * N_SB    # dest superblock, 0..N_SB-1
    d = sb * SB + r_of[i]           # local dest slot, 0..N_SB*SB-1

    cells = _cell_order()
    n_cell = len(cells)
    cellrank = np.empty((N_CHUNK, N_SB), dtype=np.int64)
    for rank, (cc, ss) in enumerate(cells):
        cellrank[cc, ss] = rank

    key = core * n_cell + cellrank[c, sb]
    order = np.argsort(key, kind="stable")
    key_s = key[order]
    jl_s = jl[order]
    r_s = (d[order] & (SB - 1)).astype(np.int16)

    n_groups = N_CORES * n_cell
    counts = np.bincount(key_s, minlength=n_groups).reshape(
        N_CORES, n_cell)
    cell_max = counts.max(axis=0)             # per cell rank
    t_cell = -(-cell_max // P)                # tiles per cell
    slots_cell = t_cell * P

    off_cell = np.zeros(n_cell + 1, dtype=np.int64)
    np.cumsum(slots_cell, out=off_cell[1:])
    n_slots = int(off_cell[-1])
    ktot = n_slots // P

    starts = np.zeros(n_groups, dtype=np.int64)
    np.cumsum(counts.ravel()[:-1], out=starts[1:])
    within = np.arange(len(key_s), dtype=np.int64) - starts[key_s]
    cell_of = key_s % n_cell
    slot = off_cell[cell_of] + within
    core_s = key_s // n_cell

    src_pad = np.zeros((N_CORES, n_slots), dtype=np.int16)  # pad -> row 0
    r_pad = np.full((N_CORES, n_slots), -1, dtype=np.int16)
    src_pad[core_s, slot] = jl_s
    r_pad[core_s, slot] = r_s

    idx_w = src_pad.reshape(N_CORES, n_slots // 16, 16).transpose(0, 2, 1)
    idx_w = np.ascontiguousarray(np.tile(idx_w, (1, 8, 1)))

    import ml_dtypes
    r_t = np.ascontiguousarray(
        r_pad.reshape(N_CORES, ktot, P).transpose(0, 2, 1)).astype(
            ml_dtypes.bfloat16)

    ntmax = int(t_cell.max())
    iota = np.broadcast_to(
        np.repeat(np.arange(SB), ntmax).astype(ml_dtypes.bfloat16),
        (P, SB * ntmax)).copy()

    hi = x.astype(ml_dtypes.bfloat16)
    lo = (x.astype(np.float32) - hi.astype(np.float32)).astype(
        ml_dtypes.bfloat16)
    x_dev = np.ascontiguousarray(np.concatenate([hi, lo], axis=1))

    t_csb = np.zeros((N_CHUNK, N_SB), dtype=np.int64)
    for rank, (cc, ss) in enumerate(cells):
        t_csb[cc, ss] = t_cell[rank]

    in_maps = [
        {"x": x_dev, "idx_w": idx_w[c2], "r_t": r_t[c2], "iota": iota}
        for c2 in range(N_CORES)
    ]
    return in_maps, t_csb, n_slots, (bin_of, r_of)


def kernel(x, triplet_indices, _msg_dt="bf16hl", _return_nc=False):
    x = np.asarray(x)
    triplet_indices = np.asarray(triplet_indices)

    in_maps, t_csb, n_slots, (bin_of, r_of) = _host_prep(x, triplet_indices)

    cache_key = t_csb.tobytes()
    if cache_key not in _PROG_CACHE:
        _PROG_CACHE[cache_key] = _build_program(t_csb.tolist())
    nc, ktot, n_slots2 = _PROG_CACHE[cache_key]
    assert n_slots2 == n_slots

    from concourse.bass_utils import run_bass_kernel_spmd

    res = run_bass_kernel_spmd(nc, in_maps, core_ids=list(range(N_CORES)))

    # undo the load-balancing permutation: global dest g lives at row
    # (bin % N_SB)*SB + r of core bin//N_SB
    out = np.empty((N_NODES, F), dtype=np.float32)
    core_of = bin_of // N_SB
    row_of = (bin_of - core_of * N_SB) * SB + r_of
    for c2 in range(N_CORES):
        o = res.results[c2]["out_t"]  # [128, N_BLK*F]
        o = o.reshape(P, N_BLK, F).transpose(1, 0, 2).reshape(N_BLK * P, F)
        m = core_of == c2
        out[m] = o[row_of[m]]
    if _return_nc:
        return out, nc, in_maps
    return out


# revision 20
# speedup vs baseline: 1.1653x; 1.0169x over previous
"""Trainium2 Bass kernel for gather + segment-sum message passing.

out = segment_sum(x[index_j], index_i, num_segments=N)

Strategy (no collectives): shard the OUTPUT (node) dimension across the
8 cores (12500 nodes each); x replicated in DRAM.  Host-side prep per
core:
  - triplets bucketed by (src chunk c of 25000 rows, dest superblock sb
    of 256 nodes), each (c,sb) cell padded to a multiple of 128 slots
    (uniform across cores so the SPMD program is shape-identical),
  - gather indices are chunk-local (fit int16 for the dma_gather
    custom op; pad slots point at row 0 of the chunk),
  - r[slot] = in-superblock destination offset (0..255) as bf16, -1
    for pads.
Slot layout is group-major -> chunk -> superblock, so each (group,
chunk) is one contiguous slot range = ONE dma_gather call.  The Q7
SWDGE descriptor-generation (~8.4 ns/index) is the main cost; the 4
SWDGE queues map to disjoint Q7 cpu pairs, so per-chunk gathers go on
queue c and desc-gen runs 4-way parallel.
On device, per superblock: a batched bf16 is_equal against an iota row
builds one-hot selection matrices S[msg, 256 node slots] (pad slots
match nothing); TensorE scatter-adds psum_q[node, f] += S_q.T @ msg
for the 2 psum halves, accumulated over the superblock's tiles;
ScalarE copies PSUM to an SBUF staging buffer; one contiguous DRAM
store at the end (host undoes the layout permutation).
Messages are gathered from a hi/lo bf16-split copy of x (one 256-byte
row per index); only the hi half feeds the matmul (~4e-3 relative
accuracy, well within the 2e-2 gate); PSUM accumulates in fp32.
"""

import numpy as np

N_NODES = 100000
N_TRIPLETS = 1250000
F = 64
N_CORES = 8
N_LOC = N_NODES // N_CORES  # 12500 nodes per core
P = 128
SB = 256                    # dest superblock (nodes)
N_SB = 52                   # superblocks per core (load-balanced bins)
N_BLK = N_SB * (SB // P)        # 104 psum blocks of 128
N_CHUNK = 4
CHUNK_ROWS = N_NODES // N_CHUNK  # 25000 src rows per chunk
G_SB = 4                    # superblocks per gather-call group

_PROG_CACHE = {}

# 12 groups of 4 superblocks + 2 tail groups of 2: smaller tail groups
# shorten the end-of-kernel drain (transfer+sel+matmul of the last group)
GROUPS = ([list(range(g0, g0 + G_SB)) for g0 in range(0, 48, G_SB)]
          + [[48, 49], [50, 51]])
assert sorted(s for g in GROUPS for s in g) == list(range(N_SB))


def _cell_order():
    """Canonical cell enumeration shared by host prep and builder:
    group-major -> chunk -> superblock."""
    cells = []
    for sbs in GROUPS:
        for c in range(N_CHUNK):
            for s in sbs:
                cells.append((c, s))
    return cells


def _build_program(t_csb):
    """t_csb: [N_CHUNK][N_SB] message-tile counts per cell (uniform
    across cores by construction)."""
    import concourse.tile as tile
    from concourse import bacc, mybir

    fdt = mybir.dt.float32
    bdt = mybir.dt.bfloat16
    idt16 = mybir.dt.int16
    mcols = 2 * F  # hi|lo bf16 row = 256 bytes

    cells = _cell_order()
    tile_off = {}
    run = 0
    for (c, s) in cells:
        tile_off[(c, s)] = run
        run += t_csb[c][s]
    ktot = run
    n_slots = ktot * P
    ntmax = max(max(row) for row in t_csb)

    nc = bacc.Bacc("TRN2", target_bir_lowering=False, debug=False,
                   num_devices=1, num_swdge_queues=4)

    x_ap = nc.dram_tensor("x", [N_NODES, mcols], bdt,
                          kind="ExternalInput").ap()
    idx_ap = nc.dram_tensor("idx_w", [P, n_slots // 16], idt16,
                            kind="ExternalInput").ap()
    r_ap = nc.dram_tensor("r_t", [P, ktot], bdt, kind="ExternalInput").ap()
    # iota_rep[p, n*ntmax + t] = n : per-tile-broadcast iota so the
    # is_equal runs with packed innermost APs (DVE 2x_1p mode)
    iota_ap = nc.dram_tensor("iota", [P, SB * ntmax], bdt,
                             kind="ExternalInput").ap()
    out_ap = nc.dram_tensor("out_t", [P, N_BLK * F], fdt,
                            kind="ExternalOutput").ap()

    with tile.TileContext(nc) as tc:
        with (
            tc.tile_pool(name="res", bufs=1) as res_pool,
            tc.tile_pool(name="stage", bufs=2) as stage_pool,
            tc.tile_pool(name="msg", bufs=4) as msg_pool,
            tc.tile_pool(name="sel", bufs=12) as sel_pool,
            tc.tile_pool(name="psum", bufs=3, space="PSUM") as psum_pool,
        ):
            # per-group idx loads so the first gather starts early;
            # group 0's idx goes first, r/iota after it
            idx_g = []
            for gi, sbs in enumerate(GROUPS):
                k0 = tile_off[(0, sbs[0])]
                k1 = (tile_off[(3, sbs[-1])] + t_csb[3][sbs[-1]])
                t_ig = res_pool.tile([P, (k1 - k0) * 8], idt16,
                                     name=f"idx_{gi}")
                nc.sync.dma_start(t_ig[:], idx_ap[:, k0 * 8:k1 * 8])
                idx_g.append((t_ig, k0))
                if gi == 0:
                    r_sb = res_pool.tile([P, ktot], bdt)
                    nc.sync.dma_start(r_sb[:], r_ap[:])
                    iota_sb = res_pool.tile([P, SB * ntmax], bdt)
                    nc.sync.dma_start(iota_sb[:], iota_ap[:])

            for gi, sbs in enumerate(GROUPS):
                # one gather call per (group, chunk): the group's cells
                # for that chunk are contiguous in slot space
                idx_sb, gk0 = idx_g[gi]
                gmsg = {}
                for c in range(N_CHUNK):
                    ct = sum(t_csb[c][s] for s in sbs)
                    if ct == 0:
                        continue
                    g0 = tile_off[(c, sbs[0])]
                    msg = msg_pool.tile([P, ct * mcols], bdt, tag=f"msg{c}",
                                        name=f"msg_{gi}_{c}")
                    nc.gpsimd.dma_gather(
                        msg[:].rearrange("p (t e) -> p t e", e=mcols),
                        x_ap[c * CHUNK_ROWS:(c + 1) * CHUNK_ROWS, :],
                        idx_sb[:, (g0 - gk0) * 8:(g0 - gk0 + ct) * 8],
                        ct * P,
                        ct * P,
                        mcols,
                        single_packet=False,
                        queue_num=c,
                    )
                    gmsg[c] = (msg, g0)

                blk0 = sbs[0] * (SB // P)
                nblk_g = len(sbs) * (SB // P)
                out_g = stage_pool.tile([P, nblk_g * F], fdt, tag="out",
                                        name=f"out_{gi}")
                for s in sbs:
                    n_tiles_sb = sum(t_csb[c][s] for c in range(N_CHUNK))
                    if n_tiles_sb == 0:
                        continue
                    sels = {}
                    for c in range(N_CHUNK):
                        nt = t_csb[c][s]
                        if nt == 0:
                            continue
                        g0 = tile_off[(c, s)]
                        sel = sel_pool.tile([P, SB * nt], bdt,
                                            tag="sel", name=f"sel_{s}_{c}")
                        # S[q, n, t] = (r[q, t] == n), n in [0, 256);
                        # layout [P, SB, nt]: innermost t is packed on all
                        # operands -> DVE 2x_1p fast mode
                        nc.vector.tensor_tensor(
                            out=sel[:].rearrange("p (n t) -> p n t", t=nt),
                            in0=r_sb[:, None, g0:g0 + nt].to_broadcast(
                                [P, SB, nt]),
                            in1=iota_sb[:].rearrange(
                                "p (n t) -> p n t", t=ntmax)[:, :, :nt],
                            op=mybir.AluOpType.is_equal,
                        )
                        sels[c] = sel

                    psums = [psum_pool.tile([P, F], fdt, tag=f"acc{q}",
                                            name=f"ps_{s}_{q}")
                             for q in range(SB // P)]
                    mm_i = 0
                    n_mm = n_tiles_sb
                    for c in range(N_CHUNK):
                        nt = t_csb[c][s]
                        if nt == 0:
                            continue
                        msg, mg0 = gmsg[c]
                        lt0 = tile_off[(c, s)] - mg0
                        sel = sels[c]
                        sel_r = sel[:].rearrange("p (n t) -> p n t", t=nt)
                        for t in range(nt):
                            mt = lt0 + t
                            for q in range(SB // P):
                                lhs = sel_r[:, q * P:(q + 1) * P, t]
                                nc.tensor.matmul(
                                    out=psums[q][:],
                                    lhsT=lhs,
                                    rhs=msg[:, mt * mcols:mt * mcols + F],
                                    start=(mm_i == 0),
                                    stop=(mm_i == n_mm - 1),
                                )
                            mm_i += 1
                    for q in range(SB // P):
                        blk = s * (SB // P) + q - blk0
                        nc.scalar.copy(out_g[:, blk * F:(blk + 1) * F],
                                       psums[q][:])
                nc.sync.dma_start(
                    out_ap[:, blk0 * F:(blk0 + nblk_g) * F], out_g[:])

    nc.compile()
    return nc, ktot, n_slots


def _assign_bins(i, c):
    """Greedy chunk-aware balancing of dest nodes into the
    N_CORES*N_SB (core, superblock) bins of <=SB dests each, so every
    (bin, chunk) cell count stays ~T/(bins*chunks) and the uniform
    per-cell tile count is minimal.  Returns (bin_of, r_of): for each
    dest node its bin id and its position within the bin."""
    nbins = N_CORES * N_SB
    cnt = np.bincount(i * N_CHUNK + c,
                      minlength=N_NODES * N_CHUNK).reshape(
        N_NODES, N_CHUNK).astype(np.int64)
    order = np.argsort(-cnt.sum(1), kind="stable")
    loads = np.zeros((nbins, N_CHUNK), dtype=np.int64)
    fill = np.zeros(nbins, dtype=np.int64)
    bin_of = np.empty(N_NODES, dtype=np.int64)
    r_of = np.empty(N_NODES, dtype=np.int64)
    for dest in order:
        v = cnt[dest]
        score = (loads + v).max(1) + (fill == SB) * (1 << 30)
        b = int(score.argmin())
        bin_of[dest] = b
        r_of[dest] = fill[b]
        fill[b] += 1
        loads[b] += v
    return bin_of, r_of


def _host_prep(x, triplet_indices):
    j = np.ascontiguousarray(triplet_indices[:, 1]).astype(np.int64)
    i = np.ascontiguousarray(triplet_indices[:, 2]).astype(np.int64)

    c = j // CHUNK_ROWS             # src chunk, 0..3
    jl = (j - c * CHUNK_ROWS).astype(np.int16)

    bin_of, r_of = _assign_bins(i, c)
    core = bin_of[i] // N_SB
    sb = bin_of[i] - core * N_SB    # dest superblock, 0..N_SB-1
    d = sb * SB + r_of[i]           # local dest slot, 0..N_SB*SB-1

    cells = _cell_order()
    n_cell = len(cells)
    cellrank = np.empty((N_CHUNK, N_SB), dtype=np.int64)
    for rank, (cc, ss) in enumerate(cells):
        cellrank[cc, ss] = rank

    key = core * n_cell + cellrank[c, sb]
    order = np.argsort(key, kind="stable")
    key_s = key[order]
    jl_s = jl[order]
    r_s = (d[order] & (SB - 1)).astype(np.int16)

    n_groups = N_CORES * n_cell
    counts = np.bincount(key_s, minlength=n_groups).reshape(
        N_CORES, n_cell)
    cell_max = counts.max(axis=0)             # per cell rank
    t_cell = -(-cell_max // P)                # tiles per cell
    slots_cell = t_cell * P

    off_cell = np.zeros(n_cell + 1, dtype=np.int64)
    np.cumsum(slots_cell, out=off_cell[1:])
    n_slots = int(off_cell[-1])
    ktot = n_slots // P

    starts = np.zeros(n_groups, dtype=np.int64)
    np.cumsum(counts.ravel()[:-1], out=starts[1:])
    within = np.arange(len(key_s), dtype=np.int64) - starts[key_s]
    cell_of = key_s % n_cell
    slot = off_cell[cell_of] + within
    core_s = key_s // n_cell

    src_pad = np.zeros((N_CORES, n_slots), dtype=np.int16)  # pad -> row 0
    r_pad = np.full((N_CORES, n_slots), -1, dtype=np.int16)
    src_pad[core_s, slot] = jl_s
    r_pad[core_s, slot] = r_s

    idx_w = src_pad.reshape(N_CORES, n_slots // 16, 16).transpose(0, 2, 1)
    idx_w = np.ascontiguousarray(np.tile(idx_w, (1, 8, 1)))

    import ml_dtypes
    r_t = np.ascontiguousarray(
        r_pad.reshape(N_CORES, ktot, P).transpose(0, 2, 1)).astype(
            ml_dtypes.bfloat16)

    ntmax = int(t_cell.max())
    iota = np.broadcast_to(
        np.repeat(np.arange(SB), ntmax).astype(ml_dtypes.bfloat16),
        (P, SB * ntmax)).copy()

    hi = x.astype(ml_dtypes.bfloat16)
    lo = (x.astype(np.float32) - hi.astype(np.float32)).astype(
        ml_dtypes.bfloat16)
    x_dev = np.ascontiguousarray(np.concatenate([hi, lo], axis=1))

    t_csb = np.zeros((N_CHUNK, N_SB), dtype=np.int64)
    for rank, (cc, ss) in enumerate(cells):
        t_csb[cc, ss] = t_cell[rank]

    in_maps = [
        {"x": x_dev, "idx_w": idx_w[c2], "r_t": r_t[c2], "iota": iota}
        for c2 in range(N_CORES)
    ]
    return in_maps, t_csb, n_slots, (bin_of, r_of)


def kernel(x, triplet_indices, _msg_dt="bf16hl", _return_nc=False):
    x = np.asarray(x)
    triplet_indices = np.asarray(triplet_indices)

    in_maps, t_csb, n_slots, (bin_of, r_of) = _host_prep(x, triplet_indices)

    cache_key = t_csb.tobytes()
    if cache_key not in _PROG_CACHE:
        _PROG_CACHE[cache_key] = _build_program(t_csb.tolist())
    nc, ktot, n_slots2 = _PROG_CACHE[cache_key]
    assert n_slots2 == n_slots

    from concourse.bass_utils import run_bass_kernel_spmd

    res = run_bass_kernel_spmd(nc, in_maps, core_ids=list(range(N_CORES)))

    # undo the load-balancing permutation: global dest g lives at row
    # (bin % N_SB)*SB + r of core bin//N_SB
    out = np.empty((N_NODES, F), dtype=np.float32)
    core_of = bin_of // N_SB
    row_of = (bin_of - core_of * N_SB) * SB + r_of
    for c2 in range(N_CORES):
        o = res.results[c2]["out_t"]  # [128, N_BLK*F]
        o = o.reshape(P, N_BLK, F).transpose(1, 0, 2).reshape(N_BLK * P, F)
        m = core_of == c2
        out[m] = o[row_of[m]]
    if _return_nc:
        return out, nc, in_maps
    return out


# revision 21
# speedup vs baseline: 1.1707x; 1.0047x over previous
"""Trainium2 Bass kernel for gather + segment-sum message passing.

out = segment_sum(x[index_j], index_i, num_segments=N)

Strategy (no collectives): shard the OUTPUT (node) dimension across the
8 cores; x replicated in DRAM.  The binding resource is Q7 SWDGE
descriptor generation for dma_gather (~8.4 ns/index measured): the 4
SWDGE queues map to disjoint Q7 cpu pairs (ucode dispatches on
cpu_id/2 == queue_num), so the 4 per-chunk gathers of each group run
on queues 0-3 and desc-gen runs 4-way parallel (~335 us/core for
~160k indices).

Host-side prep per core:
  - dest nodes are assigned to the 8*52 (core, superblock-of-256)
    bins by a chunk-aware greedy balancer, so every (chunk, sb) cell
    lands at <=768 messages and the uniform per-cell padding (cells
    padded to x128 slots, shared SPMD program across cores) is ~2%,
  - gather indices are chunk-local (25000-row chunks fit int16 for
    dma_gather; pad slots point at row 0),
  - r[slot] = in-superblock dest offset (0..255) as bf16, -1 for pads.
Slot layout is group-major -> chunk -> superblock; each (group, chunk)
is one contiguous slot range = ONE dma_gather call on queue c.
On device, per superblock: one bf16 is_equal against a per-tile
replicated iota builds one-hot matrices S laid out [P, 256, nt] so
every operand has a packed innermost axis and the DVE runs in 2x_1p
mode; TensorE scatter-adds psum_q[node, f] += S_q.T @ msg for the 2
psum halves; ScalarE copies PSUM to SBUF staging; per-group DRAM
stores (host undoes the balancing permutation).
Messages are gathered from a hi/lo bf16-split copy of x (one 256-byte
row per index, the dma_gather minimum); only the hi half feeds the
matmul (~2e-3 relative error, within the 2e-2 gate); PSUM accumulates
in fp32.

Measured: 1526 us (previous baseline) -> 412 us. Breakdown at 412 us:
Pool desc-gen ~371 us (saturated), DVE sel ~206 us, gather DMA
~203 us, PE ~129 us, startup ~21 us, tail ~15 us.
"""

import numpy as np

N_NODES = 100000
N_TRIPLETS = 1250000
F = 64
N_CORES = 8
N_LOC = N_NODES // N_CORES  # 12500 nodes per core
P = 128
SB = 256                    # dest superblock (nodes)
N_SB = 52                   # superblocks per core (load-balanced bins)
N_BLK = N_SB * (SB // P)        # 104 psum blocks of 128
N_CHUNK = 4
CHUNK_ROWS = N_NODES // N_CHUNK  # 25000 src rows per chunk
G_SB = 4                    # superblocks per gather-call group

_PROG_CACHE = {}

# 12 groups of 4 superblocks + 2 tail groups of 2: smaller tail groups
# shorten the end-of-kernel drain (transfer+sel+matmul of the last group)
GROUPS = ([list(range(g0, g0 + G_SB)) for g0 in range(0, 48, G_SB)]
          + [[48, 49], [50, 51]])
assert sorted(s for g in GROUPS for s in g) == list(range(N_SB))


def _cell_order():
    """Canonical cell enumeration shared by host prep and builder:
    group-major -> chunk -> superblock."""
    cells = []
    for sbs in GROUPS:
        for c in range(N_CHUNK):
            for s in sbs:
                cells.append((c, s))
    return cells


def _build_program(t_csb):
    """t_csb: [N_CHUNK][N_SB] message-tile counts per cell (uniform
    across cores by construction)."""
    import concourse.tile as tile
    from concourse import bacc, mybir

    fdt = mybir.dt.float32
    bdt = mybir.dt.bfloat16
    idt16 = mybir.dt.int16
    mcols = 2 * F  # hi|lo bf16 row = 256 bytes

    cells = _cell_order()
    tile_off = {}
    run = 0
    for (c, s) in cells:
        tile_off[(c, s)] = run
        run += t_csb[c][s]
    ktot = run
    n_slots = ktot * P
    ntmax = max(max(row) for row in t_csb)

    nc = bacc.Bacc("TRN2", target_bir_lowering=False, debug=False,
                   num_devices=1, num_swdge_queues=4)

    x_ap = nc.dram_tensor("x", [N_NODES, mcols], bdt,
                          kind="ExternalInput").ap()
    idx_ap = nc.dram_tensor("idx_w", [P, n_slots // 16], idt16,
                            kind="ExternalInput").ap()
    r_ap = nc.dram_tensor("r_t", [P, ktot], bdt, kind="ExternalInput").ap()
    # iota_rep[p, n*ntmax + t] = n : per-tile-broadcast iota so the
    # is_equal runs with packed innermost APs (DVE 2x_1p mode)
    iota_ap = nc.dram_tensor("iota", [P, SB * ntmax], bdt,
                             kind="ExternalInput").ap()
    out_ap = nc.dram_tensor("out_t", [P, N_BLK * F], fdt,
                            kind="ExternalOutput").ap()

    with tile.TileContext(nc) as tc:
        with (
            tc.tile_pool(name="res", bufs=1) as res_pool,
            tc.tile_pool(name="stage", bufs=2) as stage_pool,
            tc.tile_pool(name="msg", bufs=4) as msg_pool,
            tc.tile_pool(name="sel", bufs=12) as sel_pool,
            tc.tile_pool(name="psum", bufs=3, space="PSUM") as psum_pool,
        ):
            # per-group idx loads so the first gather starts early;
            # group 0's idx goes first, r/iota after it
            idx_g = []
            for gi, sbs in enumerate(GROUPS):
                k0 = tile_off[(0, sbs[0])]
                k1 = (tile_off[(3, sbs[-1])] + t_csb[3][sbs[-1]])
                t_ig = res_pool.tile([P, (k1 - k0) * 8], idt16,
                                     name=f"idx_{gi}")
                nc.sync.dma_start(t_ig[:], idx_ap[:, k0 * 8:k1 * 8])
                idx_g.append((t_ig, k0))
                if gi == 0:
                    r_sb = res_pool.tile([P, ktot], bdt)
                    nc.sync.dma_start(r_sb[:], r_ap[:])
                    iota_sb = res_pool.tile([P, SB * ntmax], bdt)
                    nc.sync.dma_start(iota_sb[:], iota_ap[:])

            for gi, sbs in enumerate(GROUPS):
                # one gather call per (group, chunk): the group's cells
                # for that chunk are contiguous in slot space
                idx_sb, gk0 = idx_g[gi]
                gmsg = {}
                for c in range(N_CHUNK):
                    ct = sum(t_csb[c][s] for s in sbs)
                    if ct == 0:
                        continue
                    g0 = tile_off[(c, sbs[0])]
                    msg = msg_pool.tile([P, ct * mcols], bdt, tag=f"msg{c}",
                                        name=f"msg_{gi}_{c}")
                    nc.gpsimd.dma_gather(
                        msg[:].rearrange("p (t e) -> p t e", e=mcols),
                        x_ap[c * CHUNK_ROWS:(c + 1) * CHUNK_ROWS, :],
                        idx_sb[:, (g0 - gk0) * 8:(g0 - gk0 + ct) * 8],
                        ct * P,
                        ct * P,
                        mcols,
                        single_packet=False,
                        queue_num=c,
                    )
                    gmsg[c] = (msg, g0)

                blk0 = sbs[0] * (SB // P)
                nblk_g = len(sbs) * (SB // P)
                out_g = stage_pool.tile([P, nblk_g * F], fdt, tag="out",
                                        name=f"out_{gi}")
                for s in sbs:
                    n_tiles_sb = sum(t_csb[c][s] for c in range(N_CHUNK))
                    if n_tiles_sb == 0:
                        continue
                    sels = {}
                    for c in range(N_CHUNK):
                        nt = t_csb[c][s]
                        if nt == 0:
                            continue
                        g0 = tile_off[(c, s)]
                        sel = sel_pool.tile([P, SB * nt], bdt,
                                            tag="sel", name=f"sel_{s}_{c}")
                        # S[q, n, t] = (r[q, t] == n), n in [0, 256);
                        # layout [P, SB, nt]: innermost t is packed on all
                        # operands -> DVE 2x_1p fast mode
                        nc.vector.tensor_tensor(
                            out=sel[:].rearrange("p (n t) -> p n t", t=nt),
                            in0=r_sb[:, None, g0:g0 + nt].to_broadcast(
                                [P, SB, nt]),
                            in1=iota_sb[:].rearrange(
                                "p (n t) -> p n t", t=ntmax)[:, :, :nt],
                            op=mybir.AluOpType.is_equal,
                        )
                        sels[c] = sel

                    psums = [psum_pool.tile([P, F], fdt, tag=f"acc{q}",
                                            name=f"ps_{s}_{q}")
                             for q in range(SB // P)]
                    mm_i = 0
                    n_mm = n_tiles_sb
                    for c in range(N_CHUNK):
                        nt = t_csb[c][s]
                        if nt == 0:
                            continue
                        msg, mg0 = gmsg[c]
                        lt0 = tile_off[(c, s)] - mg0
                        sel = sels[c]
                        sel_r = sel[:].rearrange("p (n t) -> p n t", t=nt)
                        for t in range(nt):
                            mt = lt0 + t
                            for q in range(SB // P):
                                lhs = sel_r[:, q * P:(q + 1) * P, t]
                                nc.tensor.matmul(
                                    out=psums[q][:],
                                    lhsT=lhs,
                                    rhs=msg[:, mt * mcols:mt * mcols + F],
                                    start=(mm_i == 0),
                                    stop=(mm_i == n_mm - 1),
                                )
                            mm_i += 1
                    for q in range(SB // P):
                        blk = s * (SB // P) + q - blk0
                        nc.scalar.copy(out_g[:, blk * F:(blk + 1) * F],
                                       psums[q][:])
                nc.sync.dma_start(
                    out_ap[:, blk0 * F:(blk0 + nblk_g) * F], out_g[:])

    nc.compile()
    return nc, ktot, n_slots


def _assign_bins(i, c):
    """Greedy chunk-aware balancing of dest nodes into the
    N_CORES*N_SB (core, superblock) bins of <=SB dests each, so every
    (bin, chunk) cell count stays ~T/(bins*chunks) and the uniform
    per-cell tile count is minimal.  Returns (bin_of, r_of): for each
    dest node its bin id and its position within the bin."""
    nbins = N_CORES * N_SB
    cnt = np.bincount(i * N_CHUNK + c,
                      minlength=N_NODES * N_CHUNK).reshape(
        N_NODES, N_CHUNK).astype(np.int64)
    order = np.argsort(-cnt.sum(1), kind="stable")
    loads = np.zeros((nbins, N_CHUNK), dtype=np.int64)
    fill = np.zeros(nbins, dtype=np.int64)
    bin_of = np.empty(N_NODES, dtype=np.int64)
    r_of = np.empty(N_NODES, dtype=np.int64)
    for dest in order:
        v = cnt[dest]
        score = (loads + v).max(1) + (fill == SB) * (1 << 30)
        b = int(score.argmin())
        bin_of[dest] = b
        r_of[dest] = fill[b]
        fill[b] += 1
        loads[b] += v
    return bin_of, r_of


def _host_prep(x, triplet_indices):
    j = np.ascontiguousarray(triplet_indices[:, 1]).astype(np.int64)
    i = np.ascontiguousarray(triplet_indices[:, 2]).astype(np.int64)

    c = j // CHUNK_ROWS             # src chunk, 0..3
    jl = (j - c * CHUNK_ROWS).astype(np.int16)

    bin_of, r_of = _assign_bins(i, c)
    core = bin_of[i] // N_SB
    sb = bin_of[i] - core * N_SB    # dest superblock, 0..N_SB-1
    d = sb * SB + r_of[i]           # local dest slot, 0..N_SB*SB-1

    cells = _cell_order()
    n_cell = len(cells)
    cellrank = np.empty((N_CHUNK, N_SB), dtype=np.int64)
    for rank, (cc, ss) in enumerate(cells):
        cellrank[cc, ss] = rank

    key = core * n_cell + cellrank[c, sb]
    order = np.argsort(key, kind="stable")
    key_s = key[order]
    jl_s = jl[order]
    r_s = (d[order] & (SB - 1)).astype(np.int16)

    n_groups = N_CORES * n_cell
    counts = np.bincount(key_s, minlength=n_groups).reshape(
        N_CORES, n_cell)
    cell_max = counts.max(axis=0)             # per cell rank
    t_cell = -(-cell_max // P)                # tiles per cell
    slots_cell = t_cell * P

    off_cell = np.zeros(n_cell + 1, dtype=np.int64)
    np.cumsum(slots_cell, out=off_cell[1:])
    n_slots = int(off_cell[-1])
    ktot = n_slots // P

    starts = np.zeros(n_groups, dtype=np.int64)
    np.cumsum(counts.ravel()[:-1], out=starts[1:])
    within = np.arange(len(key_s), dtype=np.int64) - starts[key_s]
    cell_of = key_s % n_cell
    slot = off_cell[cell_of] + within
    core_s = key_s // n_cell

    src_pad = np.zeros((N_CORES, n_slots), dtype=np.int16)  # pad -> row 0
    r_pad = np.full((N_CORES, n_slots), -1, dtype=np.int16)
    src_pad[core_s, slot] = jl_s
    r_pad[core_s, slot] = r_s

    idx_w = src_pad.reshape(N_CORES, n_slots // 16, 16).transpose(0, 2, 1)
    idx_w = np.ascontiguousarray(np.tile(idx_w, (1, 8, 1)))

    import ml_dtypes
    r_t = np.ascontiguousarray(
        r_pad.reshape(N_CORES, ktot, P).transpose(0, 2, 1)).astype(
            ml_dtypes.bfloat16)

    ntmax = int(t_cell.max())
    iota = np.broadcast_to(
        np.repeat(np.arange(SB), ntmax).astype(ml_dtypes.bfloat16),
        (P, SB * ntmax)).copy()

    hi = x.astype(ml_dtypes.bfloat16)
    lo = (x.astype(np.float32) - hi.astype(np.float32)).astype(
        ml_dtypes.bfloat16)
    x_dev = np.ascontiguousarray(np.concatenate([hi, lo], axis=1))

    t_csb = np.zeros((N_CHUNK, N_SB), dtype=np.int64)
    for rank, (cc, ss) in enumerate(cells):
        t_csb[cc, ss] = t_cell[rank]

    in_maps = [
        {"x": x_dev, "idx_w": idx_w[c2], "r_t": r_t[c2], "iota": iota}
        for c2 in range(N_CORES)
    ]
    return in_maps, t_csb, n_slots, (bin_of, r_of)


def kernel(x, triplet_indices, _msg_dt="bf16hl", _return_nc=False):
    x = np.asarray(x)
    triplet_indices = np.asarray(triplet_indices)

    in_maps, t_csb, n_slots, (bin_of, r_of) = _host_prep(x, triplet_indices)

    cache_key = t_csb.tobytes()
    if cache_key not in _PROG_CACHE:
        _PROG_CACHE[cache_key] = _build_program(t_csb.tolist())
    nc, ktot, n_slots2 = _PROG_CACHE[cache_key]
    assert n_slots2 == n_slots

    from concourse.bass_utils import run_bass_kernel_spmd

    res = run_bass_kernel_spmd(nc, in_maps, core_ids=list(range(N_CORES)))

    # undo the load-balancing permutation: global dest g lives at row
    # (bin % N_SB)*SB + r of core bin//N_SB
    out = np.empty((N_NODES, F), dtype=np.float32)
    core_of = bin_of // N_SB
    row_of = (bin_of - core_of * N_SB) * SB + r_of
    for c2 in range(N_CORES):
        o = res.results[c2]["out_t"]  # [128, N_BLK*F]
        o = o.reshape(P, N_BLK, F).transpose(1, 0, 2).reshape(N_BLK * P, F)
        m = core_of == c2
        out[m] = o[row_of[m]]
    if _return_nc:
        return out, nc, in_maps
    return out
